# revision 3
# baseline (speedup 1.0000x reference)
"""GCN layer on 8 Trainium2 cores. v11 over v7:

- The kernel is bound by dma_gather descriptor service (~2.1ns/desc,
  4-queue max, ~176ns fixed per call).  v11 merges gather calls across
  ST tiles per window (ST=2: 196 calls instead of 392) -- big calls
  (ST=7) regress (SWDGE ring capacity), small merges help.
- Same-col dedup within each (tile, bucket): duplicate cols share one
  gathered slot (their vals land in separate sval entries on the same
  lane), cutting descriptors ~1.8%.
"""

import sys

sys.path.insert(0, "/opt/trn_rl_repo")

import numpy as np

N_NODES = 100000
D = 128
LEAKY_SLOPE = 0.5
N_CORES = 8
ROWS_PER_CORE = 12500
TILE_ROWS = 128
TILES = 98
PAD_ROWS = TILES * TILE_ROWS
NB = 4
BASES = (0, 22500, 45000, 67500)
WIN = 32768
OUT_FLUSH_TILES = 7
ST = 2  # tiles per gather supertile
NST = TILES // ST

_BUILD_CACHE = {}


def _build_bass(qb: int, repeat: int = 1, nqueues: int = NB,
                do_gather: bool = True, do_compute: bool = True,
                st: int = ST):
    """qb = 128-edge chunks per (tile, bucket) segment."""
    import contextlib

    import concourse.bacc as bacc
    import concourse.mybir as mybir
    import concourse.tile as tile

    f32 = mybir.dt.float32
    i16 = mybir.dt.int16
    bf16 = mybir.dt.bfloat16

    CAP = qb * 128
    NCHUNK = NB * qb
    nst = TILES // st
    IDXS_ST = NB * st * CAP // 16

    nc = bacc.Bacc("TRN2", target_bir_lowering=False, debug=False,
                   num_devices=N_CORES, num_swdge_queues=nqueues)

    emb = nc.dram_tensor("embeds", [N_NODES, D], bf16, kind="ExternalInput")
    svals = nc.dram_tensor("svals", [TILES, 128, NCHUNK * TILE_ROWS], bf16,
                           kind="ExternalInput")
    meta16 = nc.dram_tensor("meta16", [nst, 128, IDXS_ST], i16,
                            kind="ExternalInput")
    out = nc.dram_tensor("out", [D, PAD_ROWS], f32, kind="ExternalOutput")

    ebuf_bufs = max(2, 8 // st)

    with tile.TileContext(nc) as tc:
        with (
            tc.tile_pool(name="meta", bufs=3) as meta_pool,
            tc.tile_pool(name="ebuf", bufs=ebuf_bufs) as ebuf_pool,
            tc.tile_pool(name="sval", bufs=4) as sval_pool,
            tc.tile_pool(name="evac", bufs=4) as evac_pool,
            tc.tile_pool(name="acc", bufs=2) as acc_pool,
            tc.tile_pool(name="psum", bufs=6, space="PSUM") as psum_pool,
        ):
            out_cols = OUT_FLUSH_TILES * TILE_ROWS
            if repeat > 1:
                loop_cm = tc.For_i(
                    0, repeat, 1,
                    hint_engines=(
                        mybir.EngineType.PE,
                        mybir.EngineType.DVE,
                        mybir.EngineType.Pool,
                        mybir.EngineType.SP,
                        mybir.EngineType.Activation,
                    ),
                )
            else:
                loop_cm = contextlib.nullcontext()
            with loop_cm:
                metas = {}
                ebufs = {}
                svs = {}
                accs = {}

                def issue_meta(s):
                    if s >= nst:
                        return
                    m16_s = meta_pool.tile([128, IDXS_ST], i16, tag="m16")
                    nc.sync.dma_start(out=m16_s[:], in_=meta16.ap()[s])
                    metas[s] = m16_s

                def issue_sval(t):
                    if t >= TILES:
                        return
                    sv_t = sval_pool.tile([128, NCHUNK * TILE_ROWS], bf16,
                                          tag="sv")
                    nc.sync.dma_start(out=sv_t[:], in_=svals.ap()[t])
                    svs[t] = sv_t

                def issue_gather(s):
                    if s >= nst or not do_gather:
                        return
                    m16_s = metas.pop(s)
                    e_t = ebuf_pool.tile([128, NB, st * qb, D], bf16,
                                         tag="ebuf")
                    ncols = st * CAP // 16
                    for j in range(NB):
                        hi = min(BASES[j] + WIN, N_NODES)
                        nc.gpsimd.dma_gather(
                            out_ap=e_t[:, j, :, :],
                            in_ap=emb.ap()[BASES[j]:hi, :],
                            idxs_ap=m16_s[:, j * ncols:(j + 1) * ncols],
                            num_idxs=st * CAP,
                            num_idxs_reg=st * CAP,
                            elem_size=D,
                            elem_step=D,
                            single_packet=False,
                            queue_num=j % nqueues,
                        )
                    ebufs[s] = e_t

                def issue_compute(t):
                    s, u = divmod(t, st)
                    sv_t = svs.pop(t)
                    if do_gather:
                        e_t = ebufs[s]
                        if u == st - 1:
                            del ebufs[s]
                    else:
                        e_t = ebuf_pool.tile([128, NB, st * qb, D], bf16,
                                             tag="ebuf")
                        nc.vector.memset(e_t[:, 0, 0, :], 0.0)
                    if t % OUT_FLUSH_TILES == 0:
                        acc_new = acc_pool.tile([128, out_cols], f32,
                                                tag="acc")
                        accs[0] = acc_new
                    acc_t = accs[0]
                    ps = psum_pool.tile([128, TILE_ROWS], f32, tag="ps")
                    nmm = NCHUNK if do_compute else 1
                    i = 0
                    for j in range(NB):
                        for q in range(qb):
                            if i >= nmm:
                                break
                            nc.tensor.matmul(
                                ps[:],
                                lhsT=e_t[:, j, u * qb + q, :],
                                rhs=sv_t[:, (j * qb + q) * TILE_ROWS:
                                         (j * qb + q + 1) * TILE_ROWS],
                                start=(i == 0),
                                stop=(i == nmm - 1),
                            )
                            i += 1
                        if i >= nmm:
                            break
                    col0 = (t % OUT_FLUSH_TILES) * TILE_ROWS
                    half_t = evac_pool.tile([128, TILE_ROWS], f32,
                                            tag="half")
                    nc.scalar.mul(half_t[:], ps[:], LEAKY_SLOPE)
                    nc.vector.tensor_tensor(
                        out=acc_t[:, col0:col0 + TILE_ROWS],
                        in0=ps[:],
                        in1=half_t[:],
                        op=mybir.AluOpType.max,
                    )
                    if t % OUT_FLUSH_TILES == OUT_FLUSH_TILES - 1:
                        c0 = (t - (OUT_FLUSH_TILES - 1)) * TILE_ROWS
                        nc.sync.dma_start(
                            out=out.ap()[:, c0:c0 + out_cols],
                            in_=acc_t[:],
                        )

                GLA = max(1, 6 // st)  # gather lookahead in supertiles
                for s in range(GLA + 1):
                    issue_meta(s)
                for s in range(GLA):
                    issue_gather(s)
                for t in range(3):
                    issue_sval(t)
                for s in range(nst):
                    issue_meta(s + GLA + 1)
                    issue_gather(s + GLA)
                    for u in range(st):
                        t = s * st + u
                        issue_sval(t + 3)
                        issue_compute(t)
    nc.compile()
    return nc


def _pack_rows(core, r_in_core):
    tile_of = np.empty((N_CORES, ROWS_PER_CORE), np.int64)
    rl_of = np.empty((N_CORES, ROWS_PER_CORE), np.int64)
    for c in range(N_CORES):
        cnts_r = np.bincount(r_in_core[core == c], minlength=ROWS_PER_CORE)
        order_r = np.argsort(-cnts_r, kind="stable")
        tl = np.empty(ROWS_PER_CORE, np.int64)
        sums = np.zeros(TILES, np.int64)
        ti, step = 0, 1
        for r in order_r:
            tl[r] = ti
            sums[ti] += cnts_r[r]
            nxt = ti + step
            if nxt < 0 or nxt >= TILES:
                step = -step
            else:
                ti = nxt
        for _ in range(200):
            h = int(np.argmax(sums))
            l = int(np.argmin(sums))
            gap = sums[h] - sums[l]
            if gap <= 2:
                break
            rows_h = np.where(tl == h)[0]
            rows_l = np.where(tl == l)[0]
            want = gap // 2
            dh = cnts_r[rows_h]
            dl = cnts_r[rows_l]
            diff = dh[:, None] - dl[None, :]
            good = np.abs(diff - want)
            ih, il = np.unravel_index(np.argmin(good), good.shape)
            if diff[ih, il] <= 0:
                break
            a, b = rows_h[ih], rows_l[il]
            tl[a], tl[b] = l, h
            sums[h] -= diff[ih, il]
            sums[l] += diff[ih, il]
        tile_of[c] = tl
        ordr = np.lexsort((np.arange(ROWS_PER_CORE), tl))
        pos = np.empty(ROWS_PER_CORE, np.int64)
        fills = np.zeros(TILES, np.int64)
        for r in ordr:
            pos[r] = fills[tl[r]]
            fills[tl[r]] += 1
        assert fills.max() <= TILE_ROWS
        rl_of[c] = pos
    return tile_of, rl_of


def _prep_inputs(edge_index, edge_vals, embeds):
    import ml_dtypes

    bf = ml_dtypes.bfloat16
    row = np.asarray(edge_index[0], dtype=np.int64)
    col = np.asarray(edge_index[1], dtype=np.int64)
    val = np.asarray(edge_vals, dtype=np.float32)
    embeds = np.ascontiguousarray(
        np.asarray(embeds, dtype=np.float32).astype(bf)
    )

    core = row // ROWS_PER_CORE
    r_in_core = row - core * ROWS_PER_CORE

    tile_of, rl_of = _pack_rows(core, r_in_core)
    t_idx = tile_of[core, r_in_core]
    rl = rl_of[core, r_in_core]

    tile_id = core * TILES + t_idx
    ntiles = N_CORES * TILES

    order = np.lexsort((col, tile_id))
    tsort = tile_id[order]
    csort = col[order]
    rl2 = rl[order]
    val2 = val[order]
    tcounts = np.bincount(tile_id, minlength=ntiles)
    tstarts = np.cumsum(tcounts) - tcounts

    j_sorted = np.empty(row.size, dtype=np.int64)
    for g in range(ntiles):
        s0, n = tstarts[g], tcounts[g]
        seg = csort[s0:s0 + n]
        prev = 0
        for k in range(1, NB):
            lo = np.searchsorted(seg, BASES[k])
            hi = np.searchsorted(seg, BASES[k - 1] + WIN)
            p = min(max((k * n) // NB, lo), hi)
            p = max(p, prev)
            j_sorted[s0 + prev:s0 + p] = k - 1
            prev = p
        j_sorted[s0 + prev:s0 + n] = NB - 1

    bases = np.asarray(BASES, dtype=np.int64)
    off = csort - bases[j_sorted]
    assert off.min() >= 0 and off.max() < WIN

    seg2 = tsort * NB + j_sorted
    nseg = ntiles * NB

    # same-col dedup within (tile, bucket): edges sorted by (tile, col),
    # and buckets are positional ranges, so duplicates are adjacent
    first = np.ones(seg2.size, dtype=bool)
    first[1:] = (seg2[1:] != seg2[:-1]) | (csort[1:] != csort[:-1])
    uid = np.cumsum(first) - 1  # per-edge unique-slot id
    useg = seg2[first]
    ucounts = np.bincount(useg, minlength=nseg)
    qb = max(2, int(-(-ucounts.max() // 128)))
    cap = qb * 128

    ustarts = np.cumsum(ucounts) - ucounts
    upos = np.arange(useg.size, dtype=np.int64) - ustarts[useg]
    uslots = useg * cap + upos  # slot per unique (tile, bucket, col)
    slots = uslots[uid]         # slot per edge

    n_slots = nseg * cap
    # pad slots gather a dummy row; SPREAD their indices randomly across
    # the window instead of idx 0 -- thousands of same-row reads burst at
    # segment tails serialize on one DRAM bank (measured: all-same-idx
    # gather is 5.8x slower than spread).  32000 < 32500 keeps every
    # window's dummy reads in bounds.  sval=0 nulls them regardless.
    rng = np.random.default_rng(12345)
    idx16 = rng.integers(0, 32000, n_slots).astype(np.int16)
    idx16[uslots] = off[first].astype(np.int16)

    NCHUNK = NB * qb
    sl_seg = slots // cap
    sl_t = sl_seg // NB
    sl_j = sl_seg % NB
    sl_q = (slots % cap) // 128
    sl_e = slots % 128
    sl_s = sl_j * qb + sl_q
    sv = np.zeros((ntiles, 128, NCHUNK * TILE_ROWS), dtype=np.float32)
    np.add.at(sv, (sl_t, sl_e, sl_s * TILE_ROWS + rl2), val2)
    svals = sv.astype(bf).reshape(N_CORES, TILES, 128, NCHUNK * TILE_ROWS)

    # meta16 per supertile of ST tiles, window-major:
    # call (s, j) = ST tiles' bucket-j segments concatenated,
    # wrapped in 16 partitions and replicated x8
    a = idx16.reshape(N_CORES, NST, ST, NB, qb, 8, 16)
    a = a.transpose(0, 1, 6, 3, 2, 4, 5)  # [c, s, lo, j, u, q, hi]
    a = a.reshape(N_CORES, NST, 16, NB * ST * qb * 8)
    meta16 = np.ascontiguousarray(np.tile(a, (1, 1, 8, 1)))

    colpos = tile_of * TILE_ROWS + rl_of

    return embeds, svals, meta16, colpos, qb


def _make_in_maps(embeds_np, svals, meta16, colpos):
    return [
        {"embeds": embeds_np, "svals": svals[c], "meta16": meta16[c]}
        for c in range(N_CORES)
    ]


def kernel(edge_index, edge_vals, embeds):
    from concourse.bass_utils import run_bass_kernel_spmd

    embeds_np, svals, meta16, colpos, qb = _prep_inputs(
        edge_index, edge_vals, embeds
    )

    if qb not in _BUILD_CACHE:
        _BUILD_CACHE[qb] = _build_bass(qb)
    nc = _BUILD_CACHE[qb]

    in_maps = _make_in_maps(embeds_np, svals, meta16, colpos)
    res = run_bass_kernel_spmd(nc, in_maps, core_ids=list(range(N_CORES)))

    out_full = np.empty((N_NODES, D), dtype=np.float32)
    for c in range(N_CORES):
        oc = res.results[c]["out"]  # [D, PAD_ROWS]
        out_full[c * ROWS_PER_CORE:(c + 1) * ROWS_PER_CORE] = \
            oc[:, colpos[c]].T
    return out_full


# revision 4
# speedup vs baseline: 1.0124x; 1.0124x over previous
"""GCN layer on 8 Trainium2 cores. v11 over v7:

- The kernel is bound by dma_gather descriptor service (~2.1ns/desc,
  4-queue max, ~176ns fixed per call).  v11 merges gather calls across
  ST tiles per window (ST=2: 196 calls instead of 392) -- big calls
  (ST=7) regress (SWDGE ring capacity), small merges help.
- Same-col dedup within each (tile, bucket): duplicate cols share one
  gathered slot (their vals land in separate sval entries on the same
  lane), cutting descriptors ~1.8%.
"""

import sys

sys.path.insert(0, "/opt/trn_rl_repo")

import numpy as np

N_NODES = 100000
D = 128
LEAKY_SLOPE = 0.5
N_CORES = 8
ROWS_PER_CORE = 12500
TILE_ROWS = 128
TILES = 98
PAD_ROWS = TILES * TILE_ROWS
NB = 4
BASES = (0, 22500, 45000, 67500)
WIN = 32768
OUT_FLUSH_TILES = 7
ST = 2  # tiles per gather supertile
NST = TILES // ST

_BUILD_CACHE = {}


def _build_bass(qb: int, repeat: int = 1, nqueues: int = NB,
                do_gather: bool = True, do_compute: bool = True,
                st: int = ST):
    """qb = 128-edge chunks per (tile, bucket) segment."""
    import contextlib

    import concourse.bacc as bacc
    import concourse.mybir as mybir
    import concourse.tile as tile

    f32 = mybir.dt.float32
    i16 = mybir.dt.int16
    bf16 = mybir.dt.bfloat16

    CAP = qb * 128
    NCHUNK = NB * qb
    nst = TILES // st
    IDXS_ST = NB * st * CAP // 16

    nc = bacc.Bacc("TRN2", target_bir_lowering=False, debug=False,
                   num_devices=N_CORES, num_swdge_queues=nqueues)

    emb = nc.dram_tensor("embeds", [N_NODES, D], bf16, kind="ExternalInput")
    svals = nc.dram_tensor("svals", [TILES, 128, NCHUNK * TILE_ROWS], bf16,
                           kind="ExternalInput")
    meta16 = nc.dram_tensor("meta16", [nst, 128, IDXS_ST], i16,
                            kind="ExternalInput")
    out = nc.dram_tensor("out", [D, PAD_ROWS], f32, kind="ExternalOutput")

    ebuf_bufs = max(2, 8 // st)

    with tile.TileContext(nc) as tc:
        with (
            tc.tile_pool(name="meta", bufs=3) as meta_pool,
            tc.tile_pool(name="ebuf", bufs=ebuf_bufs) as ebuf_pool,
            tc.tile_pool(name="sval", bufs=4) as sval_pool,
            tc.tile_pool(name="evac", bufs=4) as evac_pool,
            tc.tile_pool(name="acc", bufs=2) as acc_pool,
            tc.tile_pool(name="psum", bufs=6, space="PSUM") as psum_pool,
        ):
            out_cols = OUT_FLUSH_TILES * TILE_ROWS
            if repeat > 1:
                loop_cm = tc.For_i(
                    0, repeat, 1,
                    hint_engines=(
                        mybir.EngineType.PE,
                        mybir.EngineType.DVE,
                        mybir.EngineType.Pool,
                        mybir.EngineType.SP,
                        mybir.EngineType.Activation,
                    ),
                )
            else:
                loop_cm = contextlib.nullcontext()
            with loop_cm:
                metas = {}
                ebufs = {}
                svs = {}
                accs = {}

                def issue_meta(s):
                    if s >= nst:
                        return
                    m16_s = meta_pool.tile([128, IDXS_ST], i16, tag="m16")
                    nc.sync.dma_start(out=m16_s[:], in_=meta16.ap()[s])
                    metas[s] = m16_s

                def issue_sval(t):
                    if t >= TILES:
                        return
                    sv_t = sval_pool.tile([128, NCHUNK * TILE_ROWS], bf16,
                                          tag="sv")
                    nc.sync.dma_start(out=sv_t[:], in_=svals.ap()[t])
                    svs[t] = sv_t

                def issue_gather(s):
                    if s >= nst or not do_gather:
                        return
                    m16_s = metas.pop(s)
                    e_t = ebuf_pool.tile([128, NB, st * qb, D], bf16,
                                         tag="ebuf")
                    ncols = st * CAP // 16
                    for j in range(NB):
                        hi = min(BASES[j] + WIN, N_NODES)
                        nc.gpsimd.dma_gather(
                            out_ap=e_t[:, j, :, :],
                            in_ap=emb.ap()[BASES[j]:hi, :],
                            idxs_ap=m16_s[:, j * ncols:(j + 1) * ncols],
                            num_idxs=st * CAP,
                            num_idxs_reg=st * CAP,
                            elem_size=D,
                            elem_step=D,
                            single_packet=False,
                            queue_num=j % nqueues,
                        )
                    ebufs[s] = e_t

                def issue_compute(t):
                    s, u = divmod(t, st)
                    sv_t = svs.pop(t)
                    if do_gather:
                        e_t = ebufs[s]
                        if u == st - 1:
                            del ebufs[s]
                    else:
                        e_t = ebuf_pool.tile([128, NB, st * qb, D], bf16,
                                             tag="ebuf")
                        nc.vector.memset(e_t[:, 0, 0, :], 0.0)
                    if t % OUT_FLUSH_TILES == 0:
                        acc_new = acc_pool.tile([128, out_cols], f32,
                                                tag="acc")
                        accs[0] = acc_new
                    acc_t = accs[0]
                    ps = psum_pool.tile([128, TILE_ROWS], f32, tag="ps")
                    nmm = NCHUNK if do_compute else 1
                    i = 0
                    for j in range(NB):
                        for q in range(qb):
                            if i >= nmm:
                                break
                            nc.tensor.matmul(
                                ps[:],
                                lhsT=e_t[:, j, u * qb + q, :],
                                rhs=sv_t[:, (j * qb + q) * TILE_ROWS:
                                         (j * qb + q + 1) * TILE_ROWS],
                                start=(i == 0),
                                stop=(i == nmm - 1),
                            )
                            i += 1
                        if i >= nmm:
                            break
                    col0 = (t % OUT_FLUSH_TILES) * TILE_ROWS
                    half_t = evac_pool.tile([128, TILE_ROWS], f32,
                                            tag="half")
                    nc.scalar.mul(half_t[:], ps[:], LEAKY_SLOPE)
                    nc.vector.tensor_tensor(
                        out=acc_t[:, col0:col0 + TILE_ROWS],
                        in0=ps[:],
                        in1=half_t[:],
                        op=mybir.AluOpType.max,
                    )
                    if t % OUT_FLUSH_TILES == OUT_FLUSH_TILES - 1:
                        c0 = (t - (OUT_FLUSH_TILES - 1)) * TILE_ROWS
                        nc.sync.dma_start(
                            out=out.ap()[:, c0:c0 + out_cols],
                            in_=acc_t[:],
                        )

                GLA = max(1, 6 // st)  # gather lookahead in supertiles
                for s in range(GLA + 1):
                    issue_meta(s)
                for s in range(GLA):
                    issue_gather(s)
                for t in range(3):
                    issue_sval(t)
                for s in range(nst):
                    issue_meta(s + GLA + 1)
                    issue_gather(s + GLA)
                    for u in range(st):
                        t = s * st + u
                        issue_sval(t + 3)
                        issue_compute(t)
    nc.compile()
    return nc


def _pack_rows(core, r_in_core):
    tile_of = np.empty((N_CORES, ROWS_PER_CORE), np.int64)
    rl_of = np.empty((N_CORES, ROWS_PER_CORE), np.int64)
    for c in range(N_CORES):
        cnts_r = np.bincount(r_in_core[core == c], minlength=ROWS_PER_CORE)
        order_r = np.argsort(-cnts_r, kind="stable")
        tl = np.empty(ROWS_PER_CORE, np.int64)
        sums = np.zeros(TILES, np.int64)
        ti, step = 0, 1
        for r in order_r:
            tl[r] = ti
            sums[ti] += cnts_r[r]
            nxt = ti + step
            if nxt < 0 or nxt >= TILES:
                step = -step
            else:
                ti = nxt
        for _ in range(200):
            h = int(np.argmax(sums))
            l = int(np.argmin(sums))
            gap = sums[h] - sums[l]
            if gap <= 2:
                break
            rows_h = np.where(tl == h)[0]
            rows_l = np.where(tl == l)[0]
            want = gap // 2
            dh = cnts_r[rows_h]
            dl = cnts_r[rows_l]
            diff = dh[:, None] - dl[None, :]
            good = np.abs(diff - want)
            ih, il = np.unravel_index(np.argmin(good), good.shape)
            if diff[ih, il] <= 0:
                break
            a, b = rows_h[ih], rows_l[il]
            tl[a], tl[b] = l, h
            sums[h] -= diff[ih, il]
            sums[l] += diff[ih, il]
        tile_of[c] = tl
        ordr = np.lexsort((np.arange(ROWS_PER_CORE), tl))
        pos = np.empty(ROWS_PER_CORE, np.int64)
        fills = np.zeros(TILES, np.int64)
        for r in ordr:
            pos[r] = fills[tl[r]]
            fills[tl[r]] += 1
        assert fills.max() <= TILE_ROWS
        rl_of[c] = pos
    return tile_of, rl_of


def _prep_inputs(edge_index, edge_vals, embeds):
    import ml_dtypes

    bf = ml_dtypes.bfloat16
    row = np.asarray(edge_index[0], dtype=np.int64)
    col = np.asarray(edge_index[1], dtype=np.int64)
    val = np.asarray(edge_vals, dtype=np.float32)
    embeds = np.ascontiguousarray(
        np.asarray(embeds, dtype=np.float32).astype(bf)
    )

    core = row // ROWS_PER_CORE
    r_in_core = row - core * ROWS_PER_CORE

    tile_of, rl_of = _pack_rows(core, r_in_core)
    t_idx = tile_of[core, r_in_core]
    rl = rl_of[core, r_in_core]

    tile_id = core * TILES + t_idx
    ntiles = N_CORES * TILES

    order = np.lexsort((col, tile_id))
    tsort = tile_id[order]
    csort = col[order]
    rl2 = rl[order]
    val2 = val[order]
    tcounts = np.bincount(tile_id, minlength=ntiles)
    tstarts = np.cumsum(tcounts) - tcounts

    j_sorted = np.empty(row.size, dtype=np.int64)
    for g in range(ntiles):
        s0, n = tstarts[g], tcounts[g]
        seg = csort[s0:s0 + n]
        prev = 0
        for k in range(1, NB):
            lo = np.searchsorted(seg, BASES[k])
            hi = np.searchsorted(seg, BASES[k - 1] + WIN)
            p = min(max((k * n) // NB, lo), hi)
            p = max(p, prev)
            j_sorted[s0 + prev:s0 + p] = k - 1
            prev = p
        j_sorted[s0 + prev:s0 + n] = NB - 1

    bases = np.asarray(BASES, dtype=np.int64)
    off = csort - bases[j_sorted]
    assert off.min() >= 0 and off.max() < WIN

    seg2 = tsort * NB + j_sorted
    nseg = ntiles * NB

    # same-col dedup within (tile, bucket): edges sorted by (tile, col),
    # and buckets are positional ranges, so duplicates are adjacent
    first = np.ones(seg2.size, dtype=bool)
    first[1:] = (seg2[1:] != seg2[:-1]) | (csort[1:] != csort[:-1])
    uid = np.cumsum(first) - 1  # per-edge unique-slot id
    useg = seg2[first]
    ucounts = np.bincount(useg, minlength=nseg)
    qb = max(2, int(-(-ucounts.max() // 128)))
    cap = qb * 128

    ustarts = np.cumsum(ucounts) - ucounts
    upos = np.arange(useg.size, dtype=np.int64) - ustarts[useg]
    uslots = useg * cap + upos  # slot per unique (tile, bucket, col)
    slots = uslots[uid]         # slot per edge

    n_slots = nseg * cap
    # pad slots gather a dummy row; SPREAD their indices randomly across
    # the window instead of idx 0 -- thousands of same-row reads burst at
    # segment tails serialize on one DRAM bank (measured: all-same-idx
    # gather is 5.8x slower than spread).  32000 < 32500 keeps every
    # window's dummy reads in bounds.  sval=0 nulls them regardless.
    rng = np.random.default_rng(12345)
    idx16 = rng.integers(0, 32000, n_slots).astype(np.int16)
    idx16[uslots] = off[first].astype(np.int16)

    NCHUNK = NB * qb
    sl_seg = slots // cap
    sl_t = sl_seg // NB
    sl_j = sl_seg % NB
    sl_q = (slots % cap) // 128
    sl_e = slots % 128
    sl_s = sl_j * qb + sl_q
    sv = np.zeros((ntiles, 128, NCHUNK * TILE_ROWS), dtype=np.float32)
    np.add.at(sv, (sl_t, sl_e, sl_s * TILE_ROWS + rl2), val2)
    svals = sv.astype(bf).reshape(N_CORES, TILES, 128, NCHUNK * TILE_ROWS)

    # meta16 per supertile of ST tiles, window-major:
    # call (s, j) = ST tiles' bucket-j segments concatenated,
    # wrapped in 16 partitions and replicated x8
    a = idx16.reshape(N_CORES, NST, ST, NB, qb, 8, 16)
    a = a.transpose(0, 1, 6, 3, 2, 4, 5)  # [c, s, lo, j, u, q, hi]
    a = a.reshape(N_CORES, NST, 16, NB * ST * qb * 8)
    meta16 = np.ascontiguousarray(np.tile(a, (1, 1, 8, 1)))

    colpos = tile_of * TILE_ROWS + rl_of

    return embeds, svals, meta16, colpos, qb


def _make_in_maps(embeds_np, svals, meta16, colpos):
    return [
        {"embeds": embeds_np, "svals": svals[c], "meta16": meta16[c]}
        for c in range(N_CORES)
    ]


def kernel(edge_index, edge_vals, embeds):
    from concourse.bass_utils import run_bass_kernel_spmd

    embeds_np, svals, meta16, colpos, qb = _prep_inputs(
        edge_index, edge_vals, embeds
    )

    if qb not in _BUILD_CACHE:
        _BUILD_CACHE[qb] = _build_bass(qb)
    nc = _BUILD_CACHE[qb]

    in_maps = _make_in_maps(embeds_np, svals, meta16, colpos)
    try:
        res = run_bass_kernel_spmd(nc, in_maps,
                                   core_ids=list(range(N_CORES)))
    except Exception:
        # the axon device wedges transiently (~1 in 4 long runs:
        # NRT_EXEC_UNIT_UNRECOVERABLE / INTERNAL); one retry recovers it
        res = run_bass_kernel_spmd(nc, in_maps,
                                   core_ids=list(range(N_CORES)))

    out_full = np.empty((N_NODES, D), dtype=np.float32)
    for c in range(N_CORES):
        oc = res.results[c]["out"]  # [D, PAD_ROWS]
        out_full[c * ROWS_PER_CORE:(c + 1) * ROWS_PER_CORE] = \
            oc[:, colpos[c]].T
    return out_full
